# revision 5
# baseline (speedup 1.0000x reference)
"""Trainium2 Bass kernel for nn_BartEncoder_up2 (ragged sentence gather + pair cls).

Strategy (pure data parallel, 2 examples per core, 8 cores):
  - Each example's tokens occupy one contiguous row range [1, 1+T_b) of
    top_rep[b], and each sentence is a contiguous sub-slice.  So the read
    side is plain chunked DMA into packed SBUF tiles [128, 1024].
  - sen_emb [S, L, H] is produced by an indirect scatter-DMA: packed row r
    goes to output row 48*s + l (host-computed index tensor).  Rows beyond
    T_b scatter into a trash region that the host slices off.  The zero
    padding of sen_emb comes from the runner's pre-zeroed output buffers.
  - Sentence sums = segment-sum = fp32 matmul with a host-built 0/1
    indicator matrix At [128, 40] per K-tile, accumulated in PSUM.
  - cls = G @ sent_sum where G[p, s] = (mask_p/denom_p) * ((s==p0)+(s==p1)),
    host-built; masked pairs have zero rows so the matmul writes zeros.
"""

import numpy as np

import concourse.bass as bass
import concourse.bacc as bacc
import concourse.mybir as mybir
import concourse.tile as tile
from concourse.bass_utils import run_bass_kernel_spmd

B, S, P, SEQ, H = 16, 40, 512, 2048, 1024
L = 48
N_CORES = 8
EX_PER_CORE = B // N_CORES  # 2
ROWS_OUT = S * L  # 1920 valid rows per example
SEN_STRIDE = 2048  # per-example row stride in the sen output (1920 + 128 trash)
CHUNK_KT = 3  # K-tiles (of 128 rows) per DMA load chunk


_NC_CACHE: dict = {}


def _build_nc_cached(nkt: int):
    if nkt not in _NC_CACHE:
        _NC_CACHE[nkt] = _build_nc(nkt)
    return _NC_CACHE[nkt]


def _build_nc(nkt: int):
    """Build the SPMD Bass program for a given number of 128-row K-tiles
    per example (same for every core/example; data-dependence lives in the
    index/indicator input tensors)."""
    nc = bacc.Bacc("TRN2", target_bir_lowering=False)
    dt = mybir.dt

    top = nc.dram_tensor("top", [EX_PER_CORE * SEQ, H], dt.float32, kind="ExternalInput")
    at = nc.dram_tensor("at", [128, EX_PER_CORE * nkt * S], dt.float32, kind="ExternalInput")
    gt = nc.dram_tensor("gt", [S, EX_PER_CORE * P], dt.float32, kind="ExternalInput")
    sidx = nc.dram_tensor("sidx", [128, EX_PER_CORE * nkt], dt.int32, kind="ExternalInput")
    sen = nc.dram_tensor("sen", [EX_PER_CORE * SEN_STRIDE, H], dt.float32, kind="ExternalOutput")
    cls = nc.dram_tensor("cls", [EX_PER_CORE * P, H], dt.float32, kind="ExternalOutput")

    with tile.TileContext(nc) as tc:
        with (
            tc.tile_pool(name="const", bufs=1) as cpool,
            tc.tile_pool(name="tok", bufs=4) as tokpool,
            tc.tile_pool(name="stage", bufs=3) as stagepool,
            tc.tile_pool(name="ssum_ps", bufs=1, space="PSUM") as ssum_psum_pool,
            tc.tile_pool(name="cls_ps", bufs=4, space="PSUM") as cls_psum_pool,
        ):
            at_t = cpool.tile([128, EX_PER_CORE * nkt * S], dt.float32)
            gt_t = cpool.tile([S, EX_PER_CORE * P], dt.float32)
            sidx_t = cpool.tile([128, EX_PER_CORE * nkt], dt.int32)
            nc.sync.dma_start(at_t[:], at[:])
            nc.sync.dma_start(gt_t[:], gt[:])
            nc.sync.dma_start(sidx_t[:], sidx[:])

            ssum_ps = [
                ssum_psum_pool.tile([S, H], dt.float32, space="PSUM", name=f"ssum{b}", tag=f"ssum{b}")
                for b in range(EX_PER_CORE)
            ]

            for b in range(EX_PER_CORE):
                for c0 in range(0, nkt, CHUNK_KT):
                    cn = min(CHUNK_KT, nkt - c0)  # K-tiles in this chunk
                    tok = tokpool.tile([128, CHUNK_KT, H], dt.float32)
                    # rows [1 + 128*c0, 1 + 128*(c0+cn)) of example b, packed
                    # so row 128*t + p lands on partition p, free block t.
                    src = top[b * SEQ + 1 + 128 * c0 : b * SEQ + 1 + 128 * (c0 + cn), :]
                    src = src.rearrange("(t p) h -> p t h", p=128)
                    nc.sync.dma_start(tok[:, :cn, :], src)
                    for tl in range(cn):
                        t = c0 + tl
                        lhs = at_t[:, (b * nkt + t) * S : (b * nkt + t + 1) * S]
                        for h in range(2):
                            nc.tensor.matmul(
                                ssum_ps[b][:, 512 * h : 512 * (h + 1)],
                                lhsT=lhs,
                                rhs=tok[:, tl, 512 * h : 512 * (h + 1)],
                                start=(t == 0),
                                stop=(t == nkt - 1),
                            )
                        nc.gpsimd.indirect_dma_start(
                            out=sen[:],
                            out_offset=bass.IndirectOffsetOnAxis(
                                ap=sidx_t[:, b * nkt + t : b * nkt + t + 1], axis=0
                            ),
                            in_=tok[:, tl, :],
                            in_offset=None,
                        )

            for b in range(EX_PER_CORE):
                ssum = stagepool.tile([S, H], dt.float32, tag="ssum_sb")
                nc.vector.tensor_copy(ssum[:], ssum_ps[b][:])
                for ch in range(P // 128):
                    stage = stagepool.tile([128, H], dt.float32, tag="cls_stage")
                    for h in range(2):
                        cls_ps = cls_psum_pool.tile([128, 512], dt.float32, space="PSUM")
                        nc.tensor.matmul(
                            cls_ps[:],
                            lhsT=gt_t[:, b * P + 128 * ch : b * P + 128 * (ch + 1)],
                            rhs=ssum[:, 512 * h : 512 * (h + 1)],
                            start=True,
                            stop=True,
                        )
                        nc.vector.tensor_copy(stage[:, 512 * h : 512 * (h + 1)], cls_ps[:])
                    nc.sync.dma_start(
                        cls[b * P + 128 * ch : b * P + 128 * (ch + 1), :], stage[:]
                    )

    nc.compile()
    return nc


def _host_prep(sentence_length, pairs_list, passage_length, pairs_num, max_sentence_length):
    """Host-side index/indicator preparation from the small int inputs."""
    Ll = int(max_sentence_length)
    sen_mask = np.arange(S)[None, :] < passage_length[:, None]  # [B,S]
    tok = np.where(sen_mask, sentence_length - 1, 0).astype(np.int64)  # [B,S]
    cs = np.cumsum(tok, axis=1) - tok  # exclusive cumsum [B,S]
    T = tok.sum(axis=1)  # [B]
    nkt = max(1, int(-(-T.max() // 128)))  # K-tiles per example, shared
    tpad = nkt * 128

    # segment id per packed row
    r = np.arange(tpad)
    bounds = cs + tok  # [B,S] inclusive-end boundaries
    seg = np.empty((B, tpad), dtype=np.int64)
    for b in range(B):
        seg[b] = np.searchsorted(bounds[b], r, side="right")
    seg = np.clip(seg, 0, S - 1)
    valid = r[None, :] < T[:, None]  # [B,tpad]

    # At: [B, tpad, S] one-hot of seg where valid -> [B, nkt, 128, S]
    at = np.zeros((B, tpad, S), dtype=np.float32)
    bb, rr = np.nonzero(valid)
    at[bb, rr, seg[bb, rr]] = 1.0
    at = at.reshape(B, nkt, 128, S)

    # scatter dst row (within this example's SEN_STRIDE block):
    # valid: 48*seg + (r - cs[seg]); invalid: trash 1920 + (r % 128)
    l_off = r[None, :] - np.take_along_axis(cs, seg, axis=1)
    dst = np.where(valid, Ll * seg + l_off, ROWS_OUT + (r[None, :] % 128))
    dst = dst.reshape(B, nkt, 128).astype(np.int32)

    # G: [B, S, P]
    p0 = np.clip(pairs_list[:, :, 0], 0, S - 1).astype(np.int64)
    p1 = np.clip(pairs_list[:, :, 1], 0, S - 1).astype(np.int64)
    c0 = np.take_along_axis(tok, p0, axis=1).astype(np.float32)
    c1 = np.take_along_axis(tok, p1, axis=1).astype(np.float32)
    denom = np.maximum(c0 + c1, np.float32(1.0))
    pmask = (np.arange(P)[None, :] < pairs_num[:, None]).astype(np.float32)
    w = (pmask / denom).astype(np.float32)  # [B,P]
    g = np.zeros((B, S, P), dtype=np.float32)
    ppi = np.arange(P)[None, :].repeat(B, 0)
    bbi = np.arange(B)[:, None].repeat(P, 1)
    np.add.at(g, (bbi, p0, ppi), w)
    np.add.at(g, (bbi, p1, ppi), w)
    return nkt, at, dst, g


def kernel(sentence_length, pairs_list, passage_length, pairs_num, max_sentence_length, top_rep):
    sentence_length = np.asarray(sentence_length)
    pairs_list = np.asarray(pairs_list)
    passage_length = np.asarray(passage_length)
    pairs_num = np.asarray(pairs_num)
    top_np = np.asarray(top_rep)

    nkt, at, dst, g = _host_prep(
        sentence_length, pairs_list, passage_length, pairs_num, max_sentence_length
    )

    nc = _build_nc_cached(nkt)

    in_maps = []
    for c in range(N_CORES):
        ex = slice(c * EX_PER_CORE, (c + 1) * EX_PER_CORE)
        # at per core: [128, EX*nkt*S]: partition p, free (b, t, s)
        at_c = np.ascontiguousarray(
            at[ex].transpose(2, 0, 1, 3).reshape(128, EX_PER_CORE * nkt * S)
        )
        # gt per core: [S, EX*P]: partition s, free (b, p)
        gt_c = np.ascontiguousarray(g[ex].transpose(1, 0, 2).reshape(S, EX_PER_CORE * P))
        # sidx per core: [128, EX*nkt], add per-example SEN_STRIDE offset
        d = dst[ex] + (np.arange(EX_PER_CORE) * SEN_STRIDE)[:, None, None].astype(np.int32)
        sidx_c = np.ascontiguousarray(d.transpose(2, 0, 1).reshape(128, EX_PER_CORE * nkt))
        in_maps.append(
            {
                "top": top_np[ex].reshape(EX_PER_CORE * SEQ, H),
                "at": at_c,
                "gt": gt_c,
                "sidx": sidx_c.astype(np.int32),
            }
        )

    global _last_in_maps
    _last_in_maps = in_maps
    res = run_bass_kernel_spmd(nc, in_maps, core_ids=list(range(N_CORES)))

    sen_out = np.empty((B, S, L, H), dtype=np.float32)
    cls_out = np.empty((B, P, 1, H), dtype=np.float32)
    for c in range(N_CORES):
        r = res.results[c]
        sen_c = r["sen"].reshape(EX_PER_CORE, SEN_STRIDE, H)
        cls_c = r["cls"].reshape(EX_PER_CORE, P, H)
        for lb in range(EX_PER_CORE):
            gb = c * EX_PER_CORE + lb
            sen_out[gb] = sen_c[lb, :ROWS_OUT].reshape(S, L, H)
            cls_out[gb, :, 0] = cls_c[lb]
    return sen_out, cls_out


# revision 6
# speedup vs baseline: 1.0617x; 1.0617x over previous
"""Trainium2 Bass kernel for nn_BartEncoder_up2 (ragged sentence gather + pair cls).

Strategy (pure data parallel, 2 examples per core, 8 cores):
  - Each example's tokens occupy one contiguous row range [1, 1+T_b) of
    top_rep[b], and each sentence is a contiguous sub-slice.  So the read
    side is plain chunked DMA into packed SBUF tiles [128, 1024].
  - sen_emb [S, L, H] is produced by an indirect scatter-DMA: packed row r
    goes to output row 48*s + l (host-computed index tensor).  Rows beyond
    T_b scatter into a trash region that the host slices off.  The zero
    padding of sen_emb comes from the runner's pre-zeroed output buffers.
  - Sentence sums = segment-sum = fp32 matmul with a host-built 0/1
    indicator matrix At [128, 40] per K-tile, accumulated in PSUM.
  - cls = G @ sent_sum where G[p, s] = (mask_p/denom_p) * ((s==p0)+(s==p1)),
    host-built; masked pairs have zero rows so the matmul writes zeros.
"""

import numpy as np

import concourse.bass as bass
import concourse.bacc as bacc
import concourse.mybir as mybir
import concourse.tile as tile
from concourse.bass_utils import run_bass_kernel_spmd

B, S, P, SEQ, H = 16, 40, 512, 2048, 1024
L = 48
N_CORES = 8
EX_PER_CORE = B // N_CORES  # 2
ROWS_OUT = S * L  # 1920 valid rows per example
SEN_STRIDE = 2048  # per-example row stride in the sen output (1920 + 128 trash)
CHUNK_KT = 3  # K-tiles (of 128 rows) per DMA load chunk


_NC_CACHE: dict = {}


def _build_nc_cached(nkt: int):
    if nkt not in _NC_CACHE:
        _NC_CACHE[nkt] = _build_nc(nkt)
    return _NC_CACHE[nkt]


def _build_nc(nkt: int):
    """Build the SPMD Bass program for a given number of 128-row K-tiles
    per example (same for every core/example; data-dependence lives in the
    index/indicator input tensors)."""
    nc = bacc.Bacc("TRN2", target_bir_lowering=False)
    dt = mybir.dt

    top = nc.dram_tensor("top", [EX_PER_CORE * SEQ, H], dt.float32, kind="ExternalInput")
    at = nc.dram_tensor("at", [128, EX_PER_CORE * nkt * S], dt.float32, kind="ExternalInput")
    gt = nc.dram_tensor("gt", [S, EX_PER_CORE * P], dt.float32, kind="ExternalInput")
    sidx = nc.dram_tensor("sidx", [128, EX_PER_CORE * nkt], dt.int32, kind="ExternalInput")
    sen = nc.dram_tensor("sen", [EX_PER_CORE * SEN_STRIDE, H], dt.float32, kind="ExternalOutput")
    cls = nc.dram_tensor("cls", [EX_PER_CORE * P, H], dt.float32, kind="ExternalOutput")

    with tile.TileContext(nc) as tc:
        with (
            tc.tile_pool(name="const", bufs=1) as cpool,
            tc.tile_pool(name="tok", bufs=6) as tokpool,
            tc.tile_pool(name="stage", bufs=3) as stagepool,
            tc.tile_pool(name="ssum_ps", bufs=1, space="PSUM") as ssum_psum_pool,
            tc.tile_pool(name="cls_ps", bufs=4, space="PSUM") as cls_psum_pool,
        ):
            at_t = cpool.tile([128, EX_PER_CORE * nkt * S], dt.float32r)
            gt_t = cpool.tile([S, EX_PER_CORE * P], dt.float32r)
            sidx_t = cpool.tile([128, EX_PER_CORE * nkt], dt.int32)
            nc.scalar.dma_start(at_t[:], at[:].bitcast(dt.float32r))
            nc.scalar.dma_start(gt_t[:], gt[:].bitcast(dt.float32r))
            nc.scalar.dma_start(sidx_t[:], sidx[:])

            ssum_ps = [
                ssum_psum_pool.tile([S, H], dt.float32, space="PSUM", name=f"ssum{b}", tag=f"ssum{b}")
                for b in range(EX_PER_CORE)
            ]

            for b in range(EX_PER_CORE):
                for c0 in range(0, nkt, CHUNK_KT):
                    cn = min(CHUNK_KT, nkt - c0)  # K-tiles in this chunk
                    tok = tokpool.tile([128, CHUNK_KT, H], dt.float32r)
                    # rows [1 + 128*c0, 1 + 128*(c0+cn)) of example b, packed
                    # so row 128*t + p lands on partition p, free block t.
                    src = top[b * SEQ + 1 + 128 * c0 : b * SEQ + 1 + 128 * (c0 + cn), :]
                    src = src.rearrange("(t p) h -> p t h", p=128)
                    nc.sync.dma_start(tok[:, :cn, :], src.bitcast(dt.float32r))
                    for tl in range(cn):
                        t = c0 + tl
                        lhs = at_t[:, (b * nkt + t) * S : (b * nkt + t + 1) * S]
                        for h in range(2):
                            nc.tensor.matmul(
                                ssum_ps[b][:, 512 * h : 512 * (h + 1)],
                                lhsT=lhs,
                                rhs=tok[:, tl, 512 * h : 512 * (h + 1)],
                                start=(t == 0),
                                stop=(t == nkt - 1),
                            )
                        nc.gpsimd.indirect_dma_start(
                            out=sen[:],
                            out_offset=bass.IndirectOffsetOnAxis(
                                ap=sidx_t[:, b * nkt + t : b * nkt + t + 1], axis=0
                            ),
                            in_=tok[:, tl, :].bitcast(dt.float32),
                            in_offset=None,
                        )

            for b in range(EX_PER_CORE):
                ssum = stagepool.tile([S, H], dt.float32r, tag="ssum_sb")
                nc.vector.tensor_copy(ssum[:], ssum_ps[b][:])
                for ch in range(P // 128):
                    stage = stagepool.tile([128, H], dt.float32, tag="cls_stage")
                    for h in range(2):
                        cls_ps = cls_psum_pool.tile([128, 512], dt.float32, space="PSUM")
                        nc.tensor.matmul(
                            cls_ps[:],
                            lhsT=gt_t[:, b * P + 128 * ch : b * P + 128 * (ch + 1)],
                            rhs=ssum[:, 512 * h : 512 * (h + 1)],
                            start=True,
                            stop=True,
                        )
                        nc.vector.tensor_copy(stage[:, 512 * h : 512 * (h + 1)], cls_ps[:])
                    nc.scalar.dma_start(
                        cls[b * P + 128 * ch : b * P + 128 * (ch + 1), :], stage[:]
                    )

    nc.compile()
    return nc


def _host_prep(sentence_length, pairs_list, passage_length, pairs_num, max_sentence_length):
    """Host-side index/indicator preparation from the small int inputs."""
    Ll = int(max_sentence_length)
    sen_mask = np.arange(S)[None, :] < passage_length[:, None]  # [B,S]
    tok = np.where(sen_mask, sentence_length - 1, 0).astype(np.int64)  # [B,S]
    cs = np.cumsum(tok, axis=1) - tok  # exclusive cumsum [B,S]
    T = tok.sum(axis=1)  # [B]
    nkt = max(1, int(-(-T.max() // 128)))  # K-tiles per example, shared
    tpad = nkt * 128

    # segment id per packed row
    r = np.arange(tpad)
    bounds = cs + tok  # [B,S] inclusive-end boundaries
    seg = np.empty((B, tpad), dtype=np.int64)
    for b in range(B):
        seg[b] = np.searchsorted(bounds[b], r, side="right")
    seg = np.clip(seg, 0, S - 1)
    valid = r[None, :] < T[:, None]  # [B,tpad]

    # At: [B, tpad, S] one-hot of seg where valid -> [B, nkt, 128, S]
    at = np.zeros((B, tpad, S), dtype=np.float32)
    bb, rr = np.nonzero(valid)
    at[bb, rr, seg[bb, rr]] = 1.0
    at = at.reshape(B, nkt, 128, S)

    # scatter dst row (within this example's SEN_STRIDE block):
    # valid: 48*seg + (r - cs[seg]); invalid: trash 1920 + (r % 128)
    l_off = r[None, :] - np.take_along_axis(cs, seg, axis=1)
    dst = np.where(valid, Ll * seg + l_off, ROWS_OUT + (r[None, :] % 128))
    dst = dst.reshape(B, nkt, 128).astype(np.int32)

    # G: [B, S, P]
    p0 = np.clip(pairs_list[:, :, 0], 0, S - 1).astype(np.int64)
    p1 = np.clip(pairs_list[:, :, 1], 0, S - 1).astype(np.int64)
    c0 = np.take_along_axis(tok, p0, axis=1).astype(np.float32)
    c1 = np.take_along_axis(tok, p1, axis=1).astype(np.float32)
    denom = np.maximum(c0 + c1, np.float32(1.0))
    pmask = (np.arange(P)[None, :] < pairs_num[:, None]).astype(np.float32)
    w = (pmask / denom).astype(np.float32)  # [B,P]
    g = np.zeros((B, S, P), dtype=np.float32)
    ppi = np.arange(P)[None, :].repeat(B, 0)
    bbi = np.arange(B)[:, None].repeat(P, 1)
    np.add.at(g, (bbi, p0, ppi), w)
    np.add.at(g, (bbi, p1, ppi), w)
    return nkt, at, dst, g


def kernel(sentence_length, pairs_list, passage_length, pairs_num, max_sentence_length, top_rep):
    sentence_length = np.asarray(sentence_length)
    pairs_list = np.asarray(pairs_list)
    passage_length = np.asarray(passage_length)
    pairs_num = np.asarray(pairs_num)
    top_np = np.asarray(top_rep)

    nkt, at, dst, g = _host_prep(
        sentence_length, pairs_list, passage_length, pairs_num, max_sentence_length
    )

    nc = _build_nc_cached(nkt)

    in_maps = []
    for c in range(N_CORES):
        ex = slice(c * EX_PER_CORE, (c + 1) * EX_PER_CORE)
        # at per core: [128, EX*nkt*S]: partition p, free (b, t, s)
        at_c = np.ascontiguousarray(
            at[ex].transpose(2, 0, 1, 3).reshape(128, EX_PER_CORE * nkt * S)
        )
        # gt per core: [S, EX*P]: partition s, free (b, p)
        gt_c = np.ascontiguousarray(g[ex].transpose(1, 0, 2).reshape(S, EX_PER_CORE * P))
        # sidx per core: [128, EX*nkt], add per-example SEN_STRIDE offset
        d = dst[ex] + (np.arange(EX_PER_CORE) * SEN_STRIDE)[:, None, None].astype(np.int32)
        sidx_c = np.ascontiguousarray(d.transpose(2, 0, 1).reshape(128, EX_PER_CORE * nkt))
        in_maps.append(
            {
                "top": top_np[ex].reshape(EX_PER_CORE * SEQ, H),
                "at": at_c,
                "gt": gt_c,
                "sidx": sidx_c.astype(np.int32),
            }
        )

    global _last_in_maps
    _last_in_maps = in_maps
    res = run_bass_kernel_spmd(nc, in_maps, core_ids=list(range(N_CORES)))

    sen_out = np.empty((B, S, L, H), dtype=np.float32)
    cls_out = np.empty((B, P, 1, H), dtype=np.float32)
    for c in range(N_CORES):
        r = res.results[c]
        sen_c = r["sen"].reshape(EX_PER_CORE, SEN_STRIDE, H)
        cls_c = r["cls"].reshape(EX_PER_CORE, P, H)
        for lb in range(EX_PER_CORE):
            gb = c * EX_PER_CORE + lb
            sen_out[gb] = sen_c[lb, :ROWS_OUT].reshape(S, L, H)
            cls_out[gb, :, 0] = cls_c[lb]
    return sen_out, cls_out


# revision 7
# speedup vs baseline: 1.4351x; 1.3517x over previous
"""Trainium2 Bass kernel for nn_BartEncoder_up2 (ragged sentence gather + pair cls).

Strategy (pure data parallel, 2 examples per core, 8 cores):
  - Each example's tokens occupy one contiguous row range [1, 1+T_b) of
    top_rep[b], and each sentence is a contiguous sub-slice.  So the read
    side is plain chunked DMA into packed SBUF tiles [128, 1024].
  - sen_emb [S, L, H] is produced by an indirect scatter-DMA: packed row r
    goes to output row 48*s + l (host-computed index tensor).  Rows beyond
    T_b scatter into a trash region that the host slices off.  The zero
    padding of sen_emb comes from the runner's pre-zeroed output buffers.
  - Sentence sums = segment-sum = fp32 matmul with a host-built 0/1
    indicator matrix At [128, 40] per K-tile, accumulated in PSUM.
  - cls = G @ sent_sum where G[p, s] = (mask_p/denom_p) * ((s==p0)+(s==p1)),
    host-built; masked pairs have zero rows so the matmul writes zeros.
"""

import numpy as np

import concourse.bass as bass
import concourse.bacc as bacc
import concourse.mybir as mybir
import concourse.tile as tile
from concourse.bass_utils import run_bass_kernel_spmd

B, S, P, SEQ, H = 16, 40, 512, 2048, 1024
L = 48
N_CORES = 8
EX_PER_CORE = B // N_CORES  # 2
ROWS_OUT = S * L  # 1920 valid rows per example
SEN_STRIDE = 2048  # per-example row stride in the sen output (1920 + 128 trash)
CHUNK_KT = 3  # K-tiles (of 128 rows) per DMA load chunk


_NC_CACHE: dict = {}


def _build_nc_cached(nkt: int):
    if nkt not in _NC_CACHE:
        _NC_CACHE[nkt] = _build_nc(nkt)
    return _NC_CACHE[nkt]


def _build_nc(nkt: int):
    """Build the SPMD Bass program for a given number of 128-row K-tiles
    per example (same for every core/example; data-dependence lives in the
    index/indicator input tensors).

    The sen output is split into 4 DRAM tensors (example x K-tile parity) so
    the indirect scatters form 4 independent WAW chains instead of one
    serialized chain; the host sums the two disjoint parity tensors."""
    nc = bacc.Bacc("TRN2", target_bir_lowering=False)
    dt = mybir.dt

    top = nc.dram_tensor("top", [EX_PER_CORE * SEQ, H], dt.float32, kind="ExternalInput")
    at = nc.dram_tensor("at", [128, EX_PER_CORE * nkt * S], dt.float32, kind="ExternalInput")
    gt = nc.dram_tensor("gt", [S, EX_PER_CORE * P], dt.float32, kind="ExternalInput")
    sidx = nc.dram_tensor("sidx", [128, EX_PER_CORE * nkt], dt.int32, kind="ExternalInput")
    sen = [
        [
            nc.dram_tensor(f"sen{b}{par}", [SEN_STRIDE, H], dt.float32, kind="ExternalOutput")
            for par in range(2)
        ]
        for b in range(EX_PER_CORE)
    ]
    cls = nc.dram_tensor("cls", [EX_PER_CORE * P, H], dt.float32, kind="ExternalOutput")

    with tile.TileContext(nc) as tc:
        with (
            tc.tile_pool(name="const", bufs=1) as cpool,
            tc.tile_pool(name="tok", bufs=6) as tokpool,
            tc.tile_pool(name="stage", bufs=3) as stagepool,
            tc.tile_pool(name="ssum_ps", bufs=1, space="PSUM") as ssum_psum_pool,
            tc.tile_pool(name="cls_ps", bufs=4, space="PSUM") as cls_psum_pool,
        ):
            at_t = cpool.tile([128, EX_PER_CORE * nkt * S], dt.float32)
            gt_t = cpool.tile([S, EX_PER_CORE * P], dt.float32)
            sidx_t = cpool.tile([128, EX_PER_CORE * nkt], dt.int32)
            nc.scalar.dma_start(at_t[:], at[:])
            nc.scalar.dma_start(gt_t[:], gt[:])
            nc.scalar.dma_start(sidx_t[:], sidx[:])

            ssum_ps = [
                ssum_psum_pool.tile([S, H], dt.float32, space="PSUM", name=f"ssum{b}", tag=f"ssum{b}")
                for b in range(EX_PER_CORE)
            ]

            for c0 in range(0, nkt, CHUNK_KT):
                cn = min(CHUNK_KT, nkt - c0)  # K-tiles in this chunk
                for b in range(EX_PER_CORE):
                    tok = tokpool.tile([128, CHUNK_KT, H], dt.float32, name=f"tok{b}_{c0}", tag="tok")
                    # rows [1 + 128*c0, 1 + 128*(c0+cn)) of example b, packed
                    # so row 128*t + p lands on partition p, free block t.
                    src = top[b * SEQ + 1 + 128 * c0 : b * SEQ + 1 + 128 * (c0 + cn), :]
                    src = src.rearrange("(t p) h -> p t h", p=128)
                    nc.sync.dma_start(tok[:, :cn, :], src)
                    for tl in range(cn):
                        t = c0 + tl
                        lhs = at_t[:, (b * nkt + t) * S : (b * nkt + t + 1) * S]
                        for h in range(2):
                            nc.tensor.matmul(
                                ssum_ps[b][:, 512 * h : 512 * (h + 1)],
                                lhsT=lhs,
                                rhs=tok[:, tl, 512 * h : 512 * (h + 1)],
                                start=(t == 0),
                                stop=(t == nkt - 1),
                            )
                        nc.gpsimd.indirect_dma_start(
                            out=sen[b][t % 2][:],
                            out_offset=bass.IndirectOffsetOnAxis(
                                ap=sidx_t[:, b * nkt + t : b * nkt + t + 1], axis=0
                            ),
                            in_=tok[:, tl, :],
                            in_offset=None,
                        )

            for b in range(EX_PER_CORE):
                ssum = stagepool.tile([S, H], dt.float32, tag="ssum_sb")
                nc.vector.tensor_copy(ssum[:], ssum_ps[b][:])
                for ch in range(P // 128):
                    stage = stagepool.tile([128, H], dt.float32, tag="cls_stage")
                    for h in range(2):
                        cls_ps = cls_psum_pool.tile([128, 512], dt.float32, space="PSUM")
                        nc.tensor.matmul(
                            cls_ps[:],
                            lhsT=gt_t[:, b * P + 128 * ch : b * P + 128 * (ch + 1)],
                            rhs=ssum[:, 512 * h : 512 * (h + 1)],
                            start=True,
                            stop=True,
                        )
                        nc.vector.tensor_copy(stage[:, 512 * h : 512 * (h + 1)], cls_ps[:])
                    nc.scalar.dma_start(
                        cls[b * P + 128 * ch : b * P + 128 * (ch + 1), :], stage[:]
                    )

    nc.compile()
    return nc


def _host_prep(sentence_length, pairs_list, passage_length, pairs_num, max_sentence_length):
    """Host-side index/indicator preparation from the small int inputs."""
    Ll = int(max_sentence_length)
    sen_mask = np.arange(S)[None, :] < passage_length[:, None]  # [B,S]
    tok = np.where(sen_mask, sentence_length - 1, 0).astype(np.int64)  # [B,S]
    cs = np.cumsum(tok, axis=1) - tok  # exclusive cumsum [B,S]
    T = tok.sum(axis=1)  # [B]
    nkt = max(1, int(-(-T.max() // 128)))  # K-tiles per example, shared
    tpad = nkt * 128

    # segment id per packed row
    r = np.arange(tpad)
    bounds = cs + tok  # [B,S] inclusive-end boundaries
    seg = np.empty((B, tpad), dtype=np.int64)
    for b in range(B):
        seg[b] = np.searchsorted(bounds[b], r, side="right")
    seg = np.clip(seg, 0, S - 1)
    valid = r[None, :] < T[:, None]  # [B,tpad]

    # At: [B, tpad, S] one-hot of seg where valid -> [B, nkt, 128, S]
    at = np.zeros((B, tpad, S), dtype=np.float32)
    bb, rr = np.nonzero(valid)
    at[bb, rr, seg[bb, rr]] = 1.0
    at = at.reshape(B, nkt, 128, S)

    # scatter dst row (within this example's SEN_STRIDE block):
    # valid: 48*seg + (r - cs[seg]); invalid: trash 1920 + (r % 128)
    l_off = r[None, :] - np.take_along_axis(cs, seg, axis=1)
    dst = np.where(valid, Ll * seg + l_off, ROWS_OUT + (r[None, :] % 128))
    dst = dst.reshape(B, nkt, 128).astype(np.int32)

    # G: [B, S, P]
    p0 = np.clip(pairs_list[:, :, 0], 0, S - 1).astype(np.int64)
    p1 = np.clip(pairs_list[:, :, 1], 0, S - 1).astype(np.int64)
    c0 = np.take_along_axis(tok, p0, axis=1).astype(np.float32)
    c1 = np.take_along_axis(tok, p1, axis=1).astype(np.float32)
    denom = np.maximum(c0 + c1, np.float32(1.0))
    pmask = (np.arange(P)[None, :] < pairs_num[:, None]).astype(np.float32)
    w = (pmask / denom).astype(np.float32)  # [B,P]
    g = np.zeros((B, S, P), dtype=np.float32)
    ppi = np.arange(P)[None, :].repeat(B, 0)
    bbi = np.arange(B)[:, None].repeat(P, 1)
    np.add.at(g, (bbi, p0, ppi), w)
    np.add.at(g, (bbi, p1, ppi), w)
    return nkt, at, dst, g


def kernel(sentence_length, pairs_list, passage_length, pairs_num, max_sentence_length, top_rep):
    sentence_length = np.asarray(sentence_length)
    pairs_list = np.asarray(pairs_list)
    passage_length = np.asarray(passage_length)
    pairs_num = np.asarray(pairs_num)
    top_np = np.asarray(top_rep)

    nkt, at, dst, g = _host_prep(
        sentence_length, pairs_list, passage_length, pairs_num, max_sentence_length
    )

    nc = _build_nc_cached(nkt)

    in_maps = []
    for c in range(N_CORES):
        ex = slice(c * EX_PER_CORE, (c + 1) * EX_PER_CORE)
        # at per core: [128, EX*nkt*S]: partition p, free (b, t, s)
        at_c = np.ascontiguousarray(
            at[ex].transpose(2, 0, 1, 3).reshape(128, EX_PER_CORE * nkt * S)
        )
        # gt per core: [S, EX*P]: partition s, free (b, p)
        gt_c = np.ascontiguousarray(g[ex].transpose(1, 0, 2).reshape(S, EX_PER_CORE * P))
        # sidx per core: [128, EX*nkt] (per-example row, no block offset)
        sidx_c = np.ascontiguousarray(dst[ex].transpose(2, 0, 1).reshape(128, EX_PER_CORE * nkt))
        in_maps.append(
            {
                "top": top_np[ex].reshape(EX_PER_CORE * SEQ, H),
                "at": at_c,
                "gt": gt_c,
                "sidx": sidx_c.astype(np.int32),
            }
        )

    global _last_in_maps
    _last_in_maps = in_maps
    res = run_bass_kernel_spmd(nc, in_maps, core_ids=list(range(N_CORES)))

    sen_out = np.empty((B, S, L, H), dtype=np.float32)
    cls_out = np.empty((B, P, 1, H), dtype=np.float32)
    for c in range(N_CORES):
        r = res.results[c]
        cls_c = r["cls"].reshape(EX_PER_CORE, P, H)
        for lb in range(EX_PER_CORE):
            gb = c * EX_PER_CORE + lb
            merged = r[f"sen{lb}0"][:ROWS_OUT] + r[f"sen{lb}1"][:ROWS_OUT]
            sen_out[gb] = merged.reshape(S, L, H)
            cls_out[gb, :, 0] = cls_c[lb]
    return sen_out, cls_out


# revision 9
# speedup vs baseline: 1.4837x; 1.0338x over previous
"""Trainium2 Bass kernel for nn_BartEncoder_up2 (ragged sentence gather + pair cls).

Strategy (pure data parallel, 2 examples per core, 8 cores):
  - Each example's tokens occupy one contiguous row range [1, 1+T_b) of
    top_rep[b], and each sentence is a contiguous sub-slice.  So the read
    side is plain chunked DMA into packed SBUF tiles [128, 1024].
  - sen_emb [S, L, H] is produced by an indirect scatter-DMA: packed row r
    goes to output row 48*s + l (host-computed index tensor).  Rows beyond
    T_b scatter into a trash region that the host slices off.  The zero
    padding of sen_emb comes from the runner's pre-zeroed output buffers.
  - Sentence sums = segment-sum = fp32 matmul with a host-built 0/1
    indicator matrix At [128, 40] per K-tile, accumulated in PSUM.
  - cls = G @ sent_sum where G[p, s] = (mask_p/denom_p) * ((s==p0)+(s==p1)),
    host-built; masked pairs have zero rows so the matmul writes zeros.
"""

import numpy as np

import concourse.bass as bass
import concourse.bacc as bacc
import concourse.mybir as mybir
import concourse.tile as tile
from concourse.bass_utils import run_bass_kernel_spmd

B, S, P, SEQ, H = 16, 40, 512, 2048, 1024
L = 48
N_CORES = 8
EX_PER_CORE = B // N_CORES  # 2
ROWS_OUT = S * L  # 1920 valid rows per example
SEN_STRIDE = 2048  # per-example row stride in the sen output (1920 + 128 trash)
CHUNK_KT = 3  # K-tiles (of 128 rows) per DMA load chunk


_NC_CACHE: dict = {}


def _build_nc_cached(nkt: int):
    if nkt not in _NC_CACHE:
        _NC_CACHE[nkt] = _build_nc(nkt)
    return _NC_CACHE[nkt]


def _build_nc(nkt: int):
    """Build the SPMD Bass program for a given number of 128-row K-tiles
    per example (same for every core/example; data-dependence lives in the
    index/indicator input tensors).

    The sen output is split into 4 DRAM tensors (example x K-tile parity) so
    the indirect scatters form 4 independent WAW chains instead of one
    serialized chain; the host sums the two disjoint parity tensors."""
    nc = bacc.Bacc("TRN2", target_bir_lowering=False)
    dt = mybir.dt

    top = nc.dram_tensor("top", [EX_PER_CORE * SEQ, H], dt.float32, kind="ExternalInput")
    at = nc.dram_tensor("at", [128, EX_PER_CORE * nkt * S], dt.float32, kind="ExternalInput")
    gt = nc.dram_tensor("gt", [128, EX_PER_CORE * P], dt.float32, kind="ExternalInput")
    sidx = nc.dram_tensor("sidx", [128, EX_PER_CORE * nkt], dt.int32, kind="ExternalInput")
    sen = [
        [
            nc.dram_tensor(f"sen{b}{par}", [SEN_STRIDE, H], dt.float32, kind="ExternalOutput")
            for par in range(2)
        ]
        for b in range(EX_PER_CORE)
    ]
    cls = nc.dram_tensor("cls", [EX_PER_CORE * P, H], dt.float32, kind="ExternalOutput")

    with tile.TileContext(nc) as tc:
        with (
            tc.tile_pool(name="const", bufs=1) as cpool,
            tc.tile_pool(name="tok", bufs=6) as tokpool,
            tc.tile_pool(name="stage", bufs=3) as stagepool,
            tc.tile_pool(name="ssum_ps", bufs=1, space="PSUM") as ssum_psum_pool,
            tc.tile_pool(name="cls_ps", bufs=4, space="PSUM") as cls_psum_pool,
        ):
            at_t = cpool.tile([128, EX_PER_CORE * nkt * S], dt.float32)
            gt_t = cpool.tile([128, EX_PER_CORE * P], dt.float32)
            sidx_t = cpool.tile([128, EX_PER_CORE * nkt], dt.int32)
            nc.scalar.dma_start(at_t[:], at[:])
            nc.scalar.dma_start(gt_t[:], gt[:])
            nc.scalar.dma_start(sidx_t[:], sidx[:])

            # One PSUM tile holds both examples' sentence sums: rows
            # [64b, 64b+40) = example b, written via PE column-tile position
            # (0, 64b) so the two examples' matmuls run concurrently in
            # disjoint halves of the PE array.
            ssum_ps = ssum_psum_pool.tile([128, H], dt.float32, space="PSUM", name="ssum_all")

            for c0 in range(0, nkt, CHUNK_KT):
                cn = min(CHUNK_KT, nkt - c0)  # K-tiles in this chunk
                toks = []
                for b in range(EX_PER_CORE):
                    tok = tokpool.tile([128, CHUNK_KT, H], dt.float32, name=f"tok{b}_{c0}", tag="tok")
                    # rows [1 + 128*c0, 1 + 128*(c0+cn)) of example b, packed
                    # so row 128*t + p lands on partition p, free block t.
                    src = top[b * SEQ + 1 + 128 * c0 : b * SEQ + 1 + 128 * (c0 + cn), :]
                    src = src.rearrange("(t p) h -> p t h", p=128)
                    nc.sync.dma_start(tok[:, :cn, :], src)
                    toks.append(tok)
                for tl in range(cn):
                    t = c0 + tl
                    # adjacent pairs differ in col-group AND psum bank so the
                    # PE overlaps them: [(0,0) || (1,1)], then [(1,0) || (0,1)]
                    for b, h in ((0, 0), (1, 1), (1, 0), (0, 1)):
                        lhs = at_t[:, (b * nkt + t) * S : (b * nkt + t + 1) * S]
                        nc.tensor.matmul(
                            ssum_ps[64 * b : 64 * b + S, 512 * h : 512 * (h + 1)],
                            lhsT=lhs,
                            rhs=toks[b][:, tl, 512 * h : 512 * (h + 1)],
                            start=(t == 0),
                            stop=(t == nkt - 1),
                            tile_position=(0, 64 * b),
                        )
                    for b in range(EX_PER_CORE):
                        nc.gpsimd.indirect_dma_start(
                            out=sen[b][t % 2][:],
                            out_offset=bass.IndirectOffsetOnAxis(
                                ap=sidx_t[:, b * nkt + t : b * nkt + t + 1], axis=0
                            ),
                            in_=toks[b][:, tl, :],
                            in_offset=None,
                        )

            ssum = stagepool.tile([128, H], dt.float32, tag="ssum_sb")
            nc.vector.tensor_copy(ssum[:], ssum_ps[:])
            for b in range(EX_PER_CORE):
                for ch in range(P // 128):
                    stage = stagepool.tile([128, H], dt.float32, tag="cls_stage")
                    for h in range(2):
                        cls_ps = cls_psum_pool.tile([128, 512], dt.float32, space="PSUM")
                        nc.tensor.matmul(
                            cls_ps[:],
                            lhsT=gt_t[64 * b : 64 * b + S, b * P + 128 * ch : b * P + 128 * (ch + 1)],
                            rhs=ssum[64 * b : 64 * b + S, 512 * h : 512 * (h + 1)],
                            start=True,
                            stop=True,
                            tile_position=(64 * b, 0),
                        )
                        nc.vector.tensor_copy(stage[:, 512 * h : 512 * (h + 1)], cls_ps[:])
                    nc.scalar.dma_start(
                        cls[b * P + 128 * ch : b * P + 128 * (ch + 1), :], stage[:]
                    )

    nc.compile()
    return nc


def _host_prep(sentence_length, pairs_list, passage_length, pairs_num, max_sentence_length):
    """Host-side index/indicator preparation from the small int inputs."""
    Ll = int(max_sentence_length)
    sen_mask = np.arange(S)[None, :] < passage_length[:, None]  # [B,S]
    tok = np.where(sen_mask, sentence_length - 1, 0).astype(np.int64)  # [B,S]
    cs = np.cumsum(tok, axis=1) - tok  # exclusive cumsum [B,S]
    T = tok.sum(axis=1)  # [B]
    nkt = max(1, int(-(-T.max() // 128)))  # K-tiles per example, shared
    tpad = nkt * 128

    # segment id per packed row
    r = np.arange(tpad)
    bounds = cs + tok  # [B,S] inclusive-end boundaries
    seg = np.empty((B, tpad), dtype=np.int64)
    for b in range(B):
        seg[b] = np.searchsorted(bounds[b], r, side="right")
    seg = np.clip(seg, 0, S - 1)
    valid = r[None, :] < T[:, None]  # [B,tpad]

    # At: [B, tpad, S] one-hot of seg where valid -> [B, nkt, 128, S]
    at = np.zeros((B, tpad, S), dtype=np.float32)
    bb, rr = np.nonzero(valid)
    at[bb, rr, seg[bb, rr]] = 1.0
    at = at.reshape(B, nkt, 128, S)

    # scatter dst row (within this example's SEN_STRIDE block):
    # valid: 48*seg + (r - cs[seg]); invalid: trash 1920 + (r % 128)
    l_off = r[None, :] - np.take_along_axis(cs, seg, axis=1)
    dst = np.where(valid, Ll * seg + l_off, ROWS_OUT + (r[None, :] % 128))
    dst = dst.reshape(B, nkt, 128).astype(np.int32)

    # G: [B, S, P]
    p0 = np.clip(pairs_list[:, :, 0], 0, S - 1).astype(np.int64)
    p1 = np.clip(pairs_list[:, :, 1], 0, S - 1).astype(np.int64)
    c0 = np.take_along_axis(tok, p0, axis=1).astype(np.float32)
    c1 = np.take_along_axis(tok, p1, axis=1).astype(np.float32)
    denom = np.maximum(c0 + c1, np.float32(1.0))
    pmask = (np.arange(P)[None, :] < pairs_num[:, None]).astype(np.float32)
    w = (pmask / denom).astype(np.float32)  # [B,P]
    g = np.zeros((B, S, P), dtype=np.float32)
    ppi = np.arange(P)[None, :].repeat(B, 0)
    bbi = np.arange(B)[:, None].repeat(P, 1)
    np.add.at(g, (bbi, p0, ppi), w)
    np.add.at(g, (bbi, p1, ppi), w)
    return nkt, at, dst, g


def kernel(sentence_length, pairs_list, passage_length, pairs_num, max_sentence_length, top_rep):
    sentence_length = np.asarray(sentence_length)
    pairs_list = np.asarray(pairs_list)
    passage_length = np.asarray(passage_length)
    pairs_num = np.asarray(pairs_num)
    top_np = np.asarray(top_rep)

    nkt, at, dst, g = _host_prep(
        sentence_length, pairs_list, passage_length, pairs_num, max_sentence_length
    )

    nc = _build_nc_cached(nkt)

    in_maps = []
    for c in range(N_CORES):
        ex = slice(c * EX_PER_CORE, (c + 1) * EX_PER_CORE)
        # at per core: [128, EX*nkt*S]: partition p, free (b, t, s)
        at_c = np.ascontiguousarray(
            at[ex].transpose(2, 0, 1, 3).reshape(128, EX_PER_CORE * nkt * S)
        )
        # gt per core: [128, EX*P]: example b at partition rows [64b, 64b+S)
        gt_c = np.zeros((128, EX_PER_CORE * P), np.float32)
        for lb in range(EX_PER_CORE):
            gt_c[64 * lb : 64 * lb + S, lb * P : (lb + 1) * P] = g[ex][lb]
        # sidx per core: [128, EX*nkt] (per-example row, no block offset)
        sidx_c = np.ascontiguousarray(dst[ex].transpose(2, 0, 1).reshape(128, EX_PER_CORE * nkt))
        in_maps.append(
            {
                "top": top_np[ex].reshape(EX_PER_CORE * SEQ, H),
                "at": at_c,
                "gt": gt_c,
                "sidx": sidx_c.astype(np.int32),
            }
        )

    global _last_in_maps
    _last_in_maps = in_maps
    res = run_bass_kernel_spmd(nc, in_maps, core_ids=list(range(N_CORES)))

    sen_out = np.empty((B, S, L, H), dtype=np.float32)
    cls_out = np.empty((B, P, 1, H), dtype=np.float32)
    for c in range(N_CORES):
        r = res.results[c]
        cls_c = r["cls"].reshape(EX_PER_CORE, P, H)
        for lb in range(EX_PER_CORE):
            gb = c * EX_PER_CORE + lb
            merged = r[f"sen{lb}0"][:ROWS_OUT] + r[f"sen{lb}1"][:ROWS_OUT]
            sen_out[gb] = merged.reshape(S, L, H)
            cls_out[gb, :, 0] = cls_c[lb]
    return sen_out, cls_out
